# revision 12
# baseline (speedup 1.0000x reference)
"""Cumulative mean along T (running mean) for input [8, 4096, 1024] f32.

out[b, t, f] = mean(x[b, :t+1, f])

Pure data parallel over batch: 8 cores, one batch element each.

All device I/O is fp16 (rel-err tolerance 2e-2 >> the ~4e-4 fp16 error
here): the host casts inputs f32->f16 and the result f16->f32, halving
both DMA directions vs f32 (16.8 MiB/core total; ~47 us at the 358 GB/s
HBM-per-NeuronCore limit, i.e. a ~1.46 us floor per 128-row block).

Per core, blocked prefix-sum along T in 128-row blocks (measured steady
state ~1.51 us/block, PE-paced; the PE runs at 1.2 GHz effective - power
throttled with all 8 cores + DMA active - so an N=512 stream is ~427 ns):

  - main matmul per block: fp16 triangular-ones stationary x fp16 input
    -> f32 PSUM, psum[t] = local prefix(t); 2 x (N=512) streams.
  - carry chain (the only serial dependency) on VectorE in fp16: carry
    tile [64, 512]; rows 0:32 hold the carry for columns 0:512, rows
    32:64 for columns 512:1024 (row 31 / row 63 meaningful). Both hops
    read psum rows 96:128 (32-aligned partition bases; DVE cost is
    free-size-bound, ~690 ns per hop with a PSUM operand).
  - carry applied by K=32 selector-broadcast matmuls (stationary row 31
    resp. 63 all-ones) accumulating into the main PSUM banks. The two
    halves sit at PE row groups (0,0) and (32,0) via explicit
    tile_position, so the pair runs as ONE ~540 ns PE slot. fp16 (not
    f32r) stationaries matter: f32r cannot use the separate LDWEIGHTS
    path, which costs ~185 ns/block of unhidden inline weight loads.
  - software pipelining: group g's sels, scales and output DMA are
    emitted after group g+1's mains (PSUM: 2+2 blocks in flight = all
    8 banks).
  - per-row 1/(t+1) scale on ScalarE (Identity activation with a
    per-partition reciprocal column), writing fp16 output tiles.

DMA: one 512 KiB transfer per 2-block group each direction (1 MiB
batches make the pipeline bursty - one completion semaphore gates 4
blocks). Inputs on the Sync HWDGE ring, outputs on the GpSimd ring;
the first group arrives as two 256 KiB single-block DMAs and the last
group drains with per-block DMAs. Partition-subset/offset output APs
would collapse write bandwidth - keep output DMAs full-partition.

Measured (8-core SPMD, core-0 NTFF): 68.8-69.8 us in the fast clock
state, ~79-81 us when the process lands on a throttled chip (the state
is constant within a process, random across processes)."""

import numpy as np

import concourse.bacc as bacc
import concourse.tile as tile
from concourse import mybir
from concourse.bass_utils import run_bass_kernel_spmd

B, T, F = 8, 4096, 1024
P = 128
NBLK = T // P  # 32
FH = 512       # one PSUM bank of f32
NHALF = F // FH
CPG = 2        # blocks per pipeline stage

F16 = mybir.dt.float16
F32 = mybir.dt.float32
F32R = mybir.dt.float32r


def _build():
    nc = bacc.Bacc(None, target_bir_lowering=False)
    x_dram = nc.dram_tensor("x", [T, F], F16, kind="ExternalInput")
    out_dram = nc.dram_tensor("out", [T, F], F16, kind="ExternalOutput")

    lt_np = np.triu(np.ones((P, P), dtype=np.float16))  # lt[s,t]=1 for s<=t
    sel_np = np.zeros((64, P), dtype=np.float16)        # row-group selectors
    sel_np[31, :] = 1.0
    sel_np[63, :] = 1.0
    recip_np = np.ascontiguousarray(
        (1.0 / (np.arange(1, T + 1, dtype=np.float64))).astype(np.float32)
        .reshape(NBLK, P).T
    )  # [p, i] = 1/(i*128+p+1)
    lt_dram = nc.inline_tensor(lt_np, "lt_const")
    sel_dram = nc.inline_tensor(sel_np, "sel_const")
    recip_dram = nc.inline_tensor(recip_np, "recip_const")

    x_rot = x_dram.rearrange("(n p) f -> p n f", p=P)
    out_rot = out_dram.rearrange("(n p) f -> p n f", p=P)

    with tile.TileContext(nc) as tc:
        with (
            tc.tile_pool(name="const", bufs=1) as cpool,
            tc.tile_pool(name="xin", bufs=6) as xpool,
            tc.tile_pool(name="xout", bufs=3) as opool,
            tc.tile_pool(name="run", bufs=6) as rpool,
            tc.tile_pool(name="psum", bufs=4, space="PSUM") as ppool,
        ):
            lt = cpool.tile([P, P], F16)
            nc.gpsimd.dma_start(lt[:], lt_dram[:])
            sel = cpool.tile([64, P], F16)
            nc.gpsimd.dma_start(sel[:], sel_dram[:])
            recip = cpool.tile([P, NBLK], F32)
            nc.gpsimd.dma_start(recip[:], recip_dram[:])

            def flush(pend, last=False):
                psums, carries, pbase, pgsz = pend
                ot = opool.tile([P, CPG, F], F16, tag="ot")
                for c in range(pgsz):
                    if carries[c] is not None:
                        for h in range(NHALF):
                            hs = slice(h * FH, (h + 1) * FH)
                            rs = slice(32 * h, 32 * h + 32)
                            nc.tensor.matmul(
                                psums[c][:, hs], sel[rs, :], carries[c][rs, :],
                                start=False, stop=True,
                                tile_position=(32 * h, 0),
                            )
                if last:
                    for c in range(pgsz):
                        i = pbase + c
                        nc.scalar.activation(
                            ot[:, c, :], psums[c][:],
                            mybir.ActivationFunctionType.Identity,
                            scale=recip[:, i : i + 1],
                        )
                        nc.gpsimd.dma_start(
                            out_rot[:, i : i + 1, :], ot[:, c : c + 1, :]
                        )
                else:
                    for c in range(pgsz):
                        i = pbase + c
                        nc.scalar.activation(
                            ot[:, c, :], psums[c][:],
                            mybir.ActivationFunctionType.Identity,
                            scale=recip[:, i : i + 1],
                        )
                    nc.gpsimd.dma_start(
                        out_rot[:, pbase : pbase + pgsz, :], ot[:, 0:pgsz, :]
                    )

            carry = None  # [64, FH] f32r split rows, see docstring
            pend = None
            base = 0
            for g in range(NBLK // CPG):
                if g == 0:
                    xt = xpool.tile([P, CPG, F], F16, tag="xt")
                    for c in range(CPG):
                        nc.sync.dma_start(
                            xt[:, c : c + 1, :], x_rot[:, c : c + 1, :]
                        )
                else:
                    xt = xpool.tile([P, CPG, F], F16, tag="xt")
                    nc.sync.dma_start(xt[:], x_rot[:, base : base + CPG, :])

                psums = []
                carries = []
                for c in range(CPG):
                    i = base + c
                    ps = ppool.tile([P, F], F32)
                    psums.append(ps)
                    carries.append(carry)
                    for h in range(NHALF):
                        hs = slice(h * FH, (h + 1) * FH)
                        nc.tensor.matmul(
                            ps[:, hs], lt[:], xt[:, c, hs],
                            start=True, stop=(i == 0),
                        )
                    if i < NBLK - 1:
                        new_carry = rpool.tile([64, FH], F16)
                        for h in range(NHALF):
                            hs = slice(h * FH, (h + 1) * FH)
                            rs = slice(32 * h, 32 * h + 32)
                            if carry is None:
                                nc.vector.tensor_copy(
                                    new_carry[rs, :], ps[96:P, hs]
                                )
                            else:
                                nc.vector.tensor_tensor(
                                    new_carry[rs, :],
                                    carry[rs, :],
                                    ps[96:P, hs],
                                    mybir.AluOpType.add,
                                )
                        carry = new_carry

                if pend is not None:
                    flush(pend)
                pend = (psums, carries, base, CPG)
                base += CPG

            flush(pend, last=True)

    nc.compile()
    return nc


_NC_CACHE = None
last_results = None  # BassKernelResults of the most recent run (for test harness)


def kernel(inputs: np.ndarray) -> np.ndarray:
    global _NC_CACHE, last_results
    if _NC_CACHE is None:
        _NC_CACHE = _build()
    nc = _NC_CACHE
    x = np.asarray(inputs)
    assert x.shape == (B, T, F), x.shape
    x16 = np.ascontiguousarray(x.astype(np.float16))
    in_maps = [{"x": x16[b]} for b in range(B)]
    res = run_bass_kernel_spmd(nc, in_maps, core_ids=list(range(B)))
    last_results = res
    return np.stack([r["out"] for r in res.results], axis=0).astype(np.float32)
